# revision 1
# baseline (speedup 1.0000x reference)
# kernel.py — CommAwareGCN on 8 TRN2 NeuronCores (Bass/Tile, SPMD).
#
# Math (exact restructure of the reference):
#   h1 = relu(X @ W1.T + b1)          per-node           [N,128]
#   x1 = A @ h1                       edge aggregation   [N,128]
#   h2 = relu(x1 @ W2.T + b2)         per-node           [N,128]
#   p  = h2 @ Wfc.T                   per-node           [N,40]
#   out = A @ p + bfc                 edge aggregation   [N,40]
# where A[d,s] = multiplicity of edge s->d.  (gather commutes with
# per-node ops, and (A@h2)@Wfc.T == A@(h2@Wfc.T).)
#
# Sharding: nodes are packed into 128-slot blocks, blocks dealt to the 8
# cores (in-degree balanced).  Each core computes the node-level linears
# for its own slots, bf16 node tables are AllGathered, and each core
# aggregates the edges pointing into its blocks:
#   dma_gather (int16 idx) fetches h-table rows for 128-edge groups,
#   a one-hot selection matrix (DVE is_equal vs iota) scatters them with
#   TensorE matmuls accumulating into PSUM banks.
# The 51200-row table exceeds int16 range, so every gather call is split
# into two window-masked calls (rows [0,32768) and [32768,...)), using
# the verified skip-in-place semantics of negative indices; per-call
# valid counts are loaded into Pool registers from a per-core input.

import numpy as np
import ml_dtypes

BF16 = ml_dtypes.bfloat16

# ---- problem constants (hardcoded; kernel.py must be self-contained) ----
N_NODES = 50000
N_EDGES = 600000
D_IN = 128
D_HID = 128
N_CLS = 40
N_CORES = 8
P = 128
WLIM = 32768          # int16 index reach

DEFAULT_CFG = dict(
    n_nodes=N_NODES,
    n_cores=N_CORES,
    n_cls=N_CLS,
    blocks_per_core=50,   # 400 blocks * 128 slots = 51200 >= 50000
    call_groups=32,       # 128-edge groups per dma_gather call
    chunk=512,            # node-linear chunk width (PSUM free dim)
    pass_banks=4,         # PSUM banks used for aggregation accumulators
)


def _passes(cfg):
    per = cfg["pass_banks"] * 4
    bpc = cfg["blocks_per_core"]
    out = []
    left = bpc
    while left > 0:
        out.append(min(per, left))
        left -= min(per, left)
    return out


def _calls(cfg, m):
    """[(pass_id, c0, cols)] — static gather-call schedule (pass-local)."""
    CG = cfg["call_groups"]
    sched = []
    for pi, nblk in enumerate(_passes(cfg)):
        ng = nblk * m
        for c0 in range(0, ng, CG):
            sched.append((pi, c0, min(CG, ng - c0)))
    return sched


def _w1base(cfg):
    ntab = cfg["blocks_per_core"] * P * cfg["n_cores"]
    return WLIM if ntab > WLIM else max(ntab // 2, 1)


# --------------------------------------------------------------------------
# Host-side planning
# --------------------------------------------------------------------------

def _make_plan(edge_src, edge_dst, cfg):
    import heapq

    n_nodes = cfg["n_nodes"]
    n_cores = cfg["n_cores"]
    bpc = cfg["blocks_per_core"]
    npc = bpc * P
    nbins = n_cores * bpc

    src = np.asarray(edge_src).astype(np.int64).ravel()
    dst = np.asarray(edge_dst).astype(np.int64).ravel()

    deg = np.bincount(dst, minlength=n_nodes)
    order = np.argsort(-deg, kind="stable")

    # LPT: nodes (degree-descending) into the lightest bin with node space.
    bin_edges = np.zeros(nbins, dtype=np.int64)
    bin_count = np.zeros(nbins, dtype=np.int64)
    bin_of = np.empty(n_nodes, dtype=np.int64)
    slot_of = np.empty(n_nodes, dtype=np.int64)
    heap = [(0, b) for b in range(nbins)]
    heapq.heapify(heap)
    for n in order:
        while True:
            e, b = heapq.heappop(heap)
            if bin_count[b] < P and e == bin_edges[b]:
                break
        bin_of[n] = b
        slot_of[n] = bin_count[b]
        bin_count[b] += 1
        bin_edges[b] += deg[n]
        if bin_count[b] < P:
            heapq.heappush(heap, (int(bin_edges[b]), b))

    m = int(max(1, -(-int(bin_edges.max()) // P)))   # groups per block

    core_of_bin = np.arange(nbins) // bpc
    block_of_bin = np.arange(nbins) % bpc
    pid_of = core_of_bin[bin_of] * npc + block_of_bin[bin_of] * P + slot_of

    # edges grouped by destination bin
    ebin = bin_of[dst]
    eorder = np.argsort(ebin, kind="stable")
    counts = np.bincount(ebin, minlength=nbins)
    starts = np.concatenate([[0], np.cumsum(counts)])

    cap = m * P
    srcs_full = np.full((nbins, cap), -1, dtype=np.int64)   # -1 = pad slot
    dsts_full = np.full((nbins, cap), -1.0, dtype=np.float32)
    es = pid_of[src[eorder]]
    ed = slot_of[dst[eorder]].astype(np.float32)
    for b in range(nbins):
        lo, hi = starts[b], starts[b + 1]
        srcs_full[b, :hi - lo] = es[lo:hi]
        dsts_full[b, :hi - lo] = ed[lo:hi]

    G = bpc * m
    w1b = _w1base(cfg)
    call_sched = _calls(cfg, m)
    ncalls = len(call_sched)
    CG = cfg["call_groups"]
    CCOLS = (CG + 1) * 8          # idx16 columns per call slot (incl ghost)

    pass_first_block = np.cumsum([0] + _passes(cfg)[:-1])

    plan_srcs, plan_dsts = [], []
    idxw = [[], []]
    cnts_all = []
    for c in range(n_cores):
        sb = srcs_full[c * bpc:(c + 1) * bpc].reshape(G, P).T   # [P, G]
        db = dsts_full[c * bpc:(c + 1) * bpc].reshape(G, P).T
        plan_srcs.append(np.ascontiguousarray(sb))
        plan_dsts.append(np.ascontiguousarray(db.astype(BF16)))

        iw = [np.full((P, ncalls * CCOLS), -1, dtype=np.int16) for _ in range(2)]
        cnt = np.zeros(2 * ncalls, dtype=np.int32)
        for k, (pi, c0, cols) in enumerate(call_sched):
            gbase = pass_first_block[pi] * m + c0
            blkcols = sb[:, gbase:gbase + cols]                 # [P, cols]
            flat = np.full((CG + 1) * P, -2, dtype=np.int64)
            flat[:cols * P] = blkcols.T.ravel()                 # i = g*128 + p
            flat[cols * P:] = -1                                # unused slots
            flat[cols * P] = -3                                 # ghost marker
            for w in range(2):
                if w == 0:
                    sel = (flat >= 0) & (flat < w1b)
                    vals = flat
                else:
                    sel = flat >= w1b
                    vals = flat - w1b
                iv = np.where(sel, vals, -1).astype(np.int16)
                iv[cols * P] = 0                                # ghost: row 0
                n_valid = int(sel.sum()) + 1
                cnt[2 * k + w] = n_valid
                # wrap into [16, NI/16] then replicate to 128 partitions
                ni = (CG + 1) * P
                wrapped = iv.reshape(ni // 16, 16).T            # [16, ni/16]
                rep = np.tile(wrapped, (8, 1))                  # [128, ni/16]
                iw[w][:, k * CCOLS:(k + 1) * CCOLS] = rep
        idxw[0].append(iw[0])
        idxw[1].append(iw[1])
        cnts_all.append(cnt.reshape(1, -1))

    return dict(m=m, G=G, npc=npc, pid_of=pid_of, dsts=plan_dsts,
                srcs=plan_srcs, idxw0=idxw[0], idxw1=idxw[1], cnts=cnts_all,
                ncalls=ncalls, ccols=CCOLS)


# --------------------------------------------------------------------------
# Device program
# --------------------------------------------------------------------------

def _build_nc(cfg, m, dbg=False):
    import concourse.bass as bass
    import concourse.bacc as bacc
    import concourse.mybir as mybir
    import concourse.tile as tile

    f32 = mybir.dt.float32
    bf16 = mybir.dt.bfloat16
    i32 = mybir.dt.int32
    i16 = mybir.dt.int16

    n_cores = cfg["n_cores"]
    bpc = cfg["blocks_per_core"]
    npc = bpc * P
    ncls = cfg["n_cls"]
    G = bpc * m
    CG = cfg["call_groups"]
    CH = cfg["chunk"]
    ntab = npc * n_cores
    rg = [list(range(n_cores))]
    w1b = _w1base(cfg)
    call_sched = _calls(cfg, m)
    ncalls = len(call_sched)
    CCOLS = (CG + 1) * 8
    passes = _passes(cfg)
    pass_first_block = np.cumsum([0] + passes[:-1])
    pass_calls = [[(k, c0, cols) for k, (pi, c0, cols) in enumerate(call_sched)
                   if pi == q] for q in range(len(passes))]

    nc = bacc.Bacc(None, num_devices=n_cores, target_bir_lowering=False)

    xt = nc.declare_dram_parameter("xt", [P, npc], f32, isOutput=False)
    w1t = nc.declare_dram_parameter("w1t", [P, P], f32, isOutput=False)
    b1 = nc.declare_dram_parameter("b1c", [P, 1], f32, isOutput=False)
    w2t = nc.declare_dram_parameter("w2t", [P, P], f32, isOutput=False)
    b2 = nc.declare_dram_parameter("b2c", [P, 1], f32, isOutput=False)
    wfct = nc.declare_dram_parameter("wfct", [P, ncls], f32, isOutput=False)
    bfc = nc.declare_dram_parameter("bfcc", [ncls, 1], f32, isOutput=False)
    iota = nc.declare_dram_parameter("iota", [P, P], bf16, isOutput=False)
    idbf = nc.declare_dram_parameter("idbf", [P, P], bf16, isOutput=False)
    idf = nc.declare_dram_parameter("idf", [P, P], f32, isOutput=False)
    srcs = nc.declare_dram_parameter("srcs", [P, G], i32, isOutput=False)
    dsts = nc.declare_dram_parameter("dsts", [P, G], bf16, isOutput=False)
    out = nc.declare_dram_parameter("out", [npc, ncls], f32, isOutput=True)

    h1_shard = nc.dram_tensor("h1_shard", [npc, P], bf16)
    h1_tab = nc.dram_tensor("h1_tab", [ntab, P], bf16)
    p_shard = nc.dram_tensor("p_shard", [npc, P], bf16)   # 40 used, 256B rows
    p_tab = nc.dram_tensor("p_tab", [ntab, P], bf16)
    if dbg:
        h1dbg = nc.declare_dram_parameter("h1dbg", [ntab, P], bf16,
                                          isOutput=True)
        pdbg = nc.declare_dram_parameter("pdbg", [ntab, P], bf16,
                                         isOutput=True)

    POOL_E = mybir.EngineType.Pool
    gsem = nc.alloc_semaphore("gsem")

    with tile.TileContext(nc) as tc:
        with (
            tc.tile_pool(name="const", bufs=1) as cpool,
            tc.tile_pool(name="xin", bufs=3) as xpool,
            tc.tile_pool(name="hrow", bufs=2) as hrpool,
            tc.tile_pool(name="meta", bufs=2) as mpool,
            tc.tile_pool(name="vg", bufs=8) as vpool,
            tc.tile_pool(name="sg", bufs=4) as spool,
            tc.tile_pool(name="mid", bufs=2) as midpool,
            tc.tile_pool(name="ps_lin", bufs=2, space="PSUM") as pslin,
            tc.tile_pool(name="ps_agg", bufs=cfg["pass_banks"],
                         space="PSUM") as psagg,
            tc.tile_pool(name="ps_tp", bufs=2, space="PSUM") as pstp,
        ):
            # ---- resident constants ----
            w1t_s = cpool.tile([P, P], f32, tag="w1t")
            nc.sync.dma_start(out=w1t_s[:], in_=w1t[:, :])
            b1_s = cpool.tile([P, 1], f32, tag="b1")
            nc.sync.dma_start(out=b1_s[:], in_=b1[:, :])
            w2t_s = cpool.tile([P, P], f32, tag="w2t")
            nc.sync.dma_start(out=w2t_s[:], in_=w2t[:, :])
            b2_s = cpool.tile([P, 1], f32, tag="b2")
            nc.sync.dma_start(out=b2_s[:], in_=b2[:, :])
            wfct_s = cpool.tile([P, ncls], f32, tag="wfct")
            nc.sync.dma_start(out=wfct_s[:], in_=wfct[:, :])
            bfc_s = cpool.tile([ncls, 1], f32, tag="bfc")
            nc.sync.dma_start(out=bfc_s[:], in_=bfc[:, :])
            iota_s = cpool.tile([P, P], bf16, tag="iota")
            nc.sync.dma_start(out=iota_s[:], in_=iota[:, :])
            idbf_s = cpool.tile([P, P], bf16, tag="idbf")
            nc.sync.dma_start(out=idbf_s[:], in_=idbf[:, :])
            idf_s = cpool.tile([P, P], f32, tag="idf")
            nc.sync.dma_start(out=idf_s[:], in_=idf[:, :])

            iota_b = iota_s[:].rearrange("p (g f) -> p g f", g=1)

            # ---- phase A: h1 = relu(X@W1.T + b1), node-major bf16 rows ----
            pos = 0
            while pos < npc:
                w = min(CH, npc - pos)
                xc = xpool.tile([P, CH], f32, tag="xc")
                nc.sync.dma_start(out=xc[:, :w], in_=xt[:, pos:pos + w])
                ps = pslin.tile([P, CH], f32, tag="lin")
                nc.tensor.matmul(out=ps[:, :w], lhsT=w1t_s[:], rhs=xc[:, :w],
                                 start=True, stop=True)
                h1f = xpool.tile([P, CH], bf16, tag="h1f")
                nc.scalar.activation(h1f[:, :w], ps[:, :w],
                                     mybir.ActivationFunctionType.Relu,
                                     bias=b1_s[:], scale=1.0)
                nj = w // P
                hrow = hrpool.tile([P, 4, P], bf16, tag="h1row")
                for j in range(nj):
                    tp = pstp.tile([P, P], bf16, tag="tp")
                    nc.tensor.transpose(out=tp[:],
                                        in_=h1f[:, j * P:(j + 1) * P],
                                        identity=idbf_s[:])
                    nc.scalar.copy(out=hrow[:, j, :], in_=tp[:])
                dview = h1_shard[pos:pos + w, :].rearrange(
                    "(j p) f -> p j f", p=P)
                nc.sync.dma_start(out=dview, in_=hrow[:, :nj, :])
                pos += w

            # ---- AllGather h1 ----
            nc.gpsimd.collective_compute(
                "AllGather", mybir.AluOpType.bypass, replica_groups=rg,
                ins=[h1_shard[:, :].opt()], outs=[h1_tab[:, :].opt()])
            if dbg:
                nc.sync.dma_start(out=h1dbg[:, :], in_=h1_tab[:, :])

            # ---- shared aggregation sweep ----
            sweep_id = [0]

            def agg_sweep(tab, feat, opart, consume_bank):
                sid = sweep_id[0]
                sweep_id[0] += 1
                """x[opart, dst] += sum_e tab[src_e][:feat] one-hot-scatter;
                per pass calls consume_bank(k, psum_tile, nb, base_block)."""
                for q, nblk in enumerate(passes):
                    g0 = int(pass_first_block[q]) * m
                    ng = nblk * m
                    dm = mpool.tile([P, G], bf16, tag="dsts")
                    nc.sync.dma_start(out=dm[:, :ng], in_=dsts[:, g0:g0 + ng])
                    sm = mpool.tile([P, G], i32, tag="srcs",
                                    name=f"sm{sid}_{q}")
                    nc.sync.dma_start(out=sm[:, :ng], in_=srcs[:, g0:g0 + ng])

                    nbank = -(-nblk // 4)
                    aggs = [psagg.tile([opart, 4 * P], f32, tag="agg",
                                       name=f"agg{sid}_{q}_{k}")
                            for k in range(nbank)]

                    for s0 in range(0, ng, 8):
                        sn = min(8, ng - s0)
                        st = spool.tile([P, 8, P], bf16, tag="s1")
                        nc.vector.tensor_tensor(
                            out=st[:, :sn, :],
                            in0=dm[:, s0:s0 + sn].to_broadcast([P, sn, P]),
                            in1=iota_b.to_broadcast([P, sn, P]),
                            op=mybir.AluOpType.is_equal)
                        for qq in range(sn):
                            g = s0 + qq
                            bb = g // m
                            sub = g % m
                            bank, slot = bb // 4, bb % 4
                            v = vpool.tile([P, P], bf16, tag="v",
                                           name=f"v{sid}_{q}_{g}")
                            nc.gpsimd.indirect_dma_start(
                                out=v[:, :], out_offset=None,
                                in_=tab[:, :],
                                in_offset=bass.IndirectOffsetOnAxis(
                                    ap=sm[:, g:g + 1], axis=0))
                            o = aggs[bank][:, slot * P:(slot + 1) * P]
                            nc.tensor.matmul(
                                out=o, lhsT=v[:, :feat],
                                rhs=st[:, qq, :],
                                start=(sub == 0), stop=(sub == m - 1))

                    for k in range(nbank):
                        nb = min(4, nblk - 4 * k)
                        consume_bank(aggs[k], nb,
                                     int(pass_first_block[q]) + 4 * k)

            # ---- phase C: x1 -> h2 -> p rows ----
            def consume_c(agg, nb, base_block):
                wk = nb * P
                x1 = midpool.tile([P, 4 * P], f32, tag="x1")
                nc.vector.tensor_copy(out=x1[:, :wk], in_=agg[:, :wk])
                ps2 = pslin.tile([P, CH], f32, tag="lin")
                nc.tensor.matmul(out=ps2[:, :wk], lhsT=w2t_s[:],
                                 rhs=x1[:, :wk], start=True, stop=True)
                h2 = midpool.tile([P, 4 * P], f32, tag="h2")
                nc.scalar.activation(h2[:, :wk], ps2[:, :wk],
                                     mybir.ActivationFunctionType.Relu,
                                     bias=b2_s[:], scale=1.0)
                ps3 = pslin.tile([ncls, CH], f32, tag="lin")
                nc.tensor.matmul(out=ps3[:, :wk], lhsT=wfct_s[:],
                                 rhs=h2[:, :wk], start=True, stop=True)
                pbf = midpool.tile([ncls, 4 * P], bf16, tag="pbf")
                nc.scalar.copy(out=pbf[:, :wk], in_=ps3[:, :wk])
                prow = hrpool.tile([P, 4, P], bf16, tag="prow")
                for j in range(nb):
                    tp = pstp.tile([P, P], bf16, tag="tp")
                    nc.tensor.transpose(out=tp[:, :ncls],
                                        in_=pbf[:, j * P:(j + 1) * P],
                                        identity=idbf_s[:ncls, :ncls])
                    nc.scalar.copy(out=prow[:, j, :ncls], in_=tp[:, :ncls])
                base = base_block * P
                dview = p_shard[base:base + wk, :].rearrange(
                    "(j p) f -> p j f", p=P)
                nc.sync.dma_start(out=dview, in_=prow[:, :nb, :])

            agg_sweep(h1_tab, P, P, consume_c)

            # ---- AllGather p ----
            nc.gpsimd.collective_compute(
                "AllGather", mybir.AluOpType.bypass, replica_groups=rg,
                ins=[p_shard[:, :].opt()], outs=[p_tab[:, :].opt()])
            if dbg:
                nc.sync.dma_start(out=pdbg[:, :], in_=p_tab[:, :])

            # ---- phase E: out = A@p + bfc ----
            def consume_e(agg, nb, base_block):
                wk = nb * P
                oc = midpool.tile([ncls, 4 * P], f32, tag="oc")
                nc.vector.tensor_tensor(
                    out=oc[:, :wk], in0=agg[:ncls, :wk],
                    in1=bfc_s[:].to_broadcast([ncls, wk]),
                    op=mybir.AluOpType.add)
                orow = hrpool.tile([P, 4, ncls], f32, tag="orow")
                for j in range(nb):
                    tp = pstp.tile([P, P], f32, tag="tp")
                    nc.tensor.transpose(out=tp[:, :ncls],
                                        in_=oc[:, j * P:(j + 1) * P],
                                        identity=idf_s[:ncls, :ncls])
                    nc.scalar.copy(out=orow[:, j, :], in_=tp[:, :ncls])
                base = base_block * P
                dview = out[base:base + wk, :].rearrange(
                    "(j p) f -> p j f", p=P)
                nc.sync.dma_start(out=dview, in_=orow[:, :nb, :])

            agg_sweep(p_tab, ncls, ncls, consume_e)

    nc.finalize()
    return nc


# --------------------------------------------------------------------------
# Entry point
# --------------------------------------------------------------------------

def _make_in_maps(inputs, cfg, plan):
    node_features = np.asarray(inputs["node_features"], dtype=np.float32)
    W1 = np.asarray(inputs["W1"], dtype=np.float32)
    b1 = np.asarray(inputs["b1"], dtype=np.float32)
    W2 = np.asarray(inputs["W2"], dtype=np.float32)
    b2 = np.asarray(inputs["b2"], dtype=np.float32)
    Wfc = np.asarray(inputs["Wfc"], dtype=np.float32)
    bfc = np.asarray(inputs["bfc"], dtype=np.float32)

    n_nodes = cfg["n_nodes"]
    n_cores = cfg["n_cores"]
    ncls = cfg["n_cls"]
    X = node_features.reshape(n_nodes, -1)
    npc = plan["npc"]
    pid_of = plan["pid_of"]

    Xp = np.zeros((n_cores * npc, P), dtype=np.float32)
    Xp[pid_of] = X

    iota = np.broadcast_to(np.arange(P, dtype=np.float32), (P, P))
    iota = np.ascontiguousarray(iota).astype(BF16)
    ident = np.eye(P, dtype=np.float32)

    in_maps = []
    for c in range(n_cores):
        in_maps.append({
            "xt": np.ascontiguousarray(Xp[c * npc:(c + 1) * npc].T),
            "w1t": np.ascontiguousarray(W1.T),
            "b1c": b1.reshape(P, 1).copy(),
            "w2t": np.ascontiguousarray(W2.T),
            "b2c": b2.reshape(P, 1).copy(),
            "wfct": np.ascontiguousarray(Wfc.T),
            "bfcc": bfc.reshape(ncls, 1).copy(),
            "iota": iota,
            "idbf": ident.astype(BF16),
            "idf": ident,
            "srcs": np.maximum(plan["srcs"][c], 0).astype(np.int32),
            "dsts": plan["dsts"][c],
        })
    return in_maps


def _run(inputs, cfg, trace=False, dbg=False):
    from concourse import bass_utils

    n_nodes = cfg["n_nodes"]
    n_cores = cfg["n_cores"]
    ncls = cfg["n_cls"]
    plan = _make_plan(inputs["edge_src"], inputs["edge_dst"], cfg)
    pid_of = plan["pid_of"]
    in_maps = _make_in_maps(inputs, cfg, plan)

    nc = _build_nc(cfg, plan["m"], dbg=dbg)
    res = bass_utils.run_bass_kernel_spmd(
        nc, in_maps, core_ids=list(range(n_cores)), trace=trace)

    shards = np.concatenate([np.asarray(r["out"]) for r in res.results],
                            axis=0)
    out_full = shards[pid_of].reshape(1, n_nodes, ncls).astype(np.float32)
    return out_full, res


def kernel(**inputs) -> np.ndarray:
    out, _ = _run(inputs, DEFAULT_CFG, trace=False)
    return out

